# revision 3
# baseline (speedup 1.0000x reference)
"""Trainium2 Bass kernel for the NEUROPULS photonic-mesh transfer matrix (v8).

The crossing layers are discarded in the reference, so the 512x512 transfer
matrix is block-diagonal: 256 independent 2x2 complex chains of 256 step
matrices S_i = B1.diag(e^{ja},e^{jb}).B0.  Each of the 8 cores computes the
chain product of its 32 steps for all 256 pairs.

Layout: pairs on partitions (k = p + 128*ks), steps/components in the free
dimension.  The whole kernel is elementwise on DVE with NO transpose, PE
matmul, PSUM, or DRAM relayout bounce.  Tiles are component-major
[c8, item] with c8 = q*4 + row*2 + col (q = 0 re / 1 im), and the host
stores steps in bit-reversed order so every tree level pairs item j with
item j + I/2: A/B operands are then contiguous item slices with the item
axis innermost (stride 1), which keeps every op in the DVE 2-byte 2x mode.

  leaf:  S comps = +-(coef_e * trig) +- (coef_o * trig_swapped)
  tree:  3 levels of batched 2x2 complex products (rest on host):
           M[qb,m,qa,r,s,n] = A[qa,r,m,n] * B[qb,m,s,n]   (4 muls)
           U = M[0,:,0]-M[1,:,1];  V = M[1,:,0]+M[0,:,1]  (one op each)
           C[q] = U/V summed over m                        (one op)

All ops obey the walrus ISA limits: TensorTensor <= 3 free dims,
ScalarTensorTensor <= 2 free dims, matching operand shapes.

Host prep sends per-MMI amplitudes t,k and per-step cos/sin (f16) - the same
six parameter-derived values per (pair, step) as the raw inputs; host finish
multiplies the per-core partials and applies the in/out heater phases (as the
baseline did).
"""

import sys

sys.path.insert(0, "/opt/trn_rl_repo")

import numpy as np

N = 512
NPAIR = 256
NCORE = 8
CH = 32  # steps per core

# item position j (0..63) holds (ks, step): step bits i0..i4 = j5,j4,j3,j2,j0
# and ks = j1 -- so level-l of the tree pairs items differing in step bit l-1,
# the final 4 items are (q? no) (ks=j1, chain-half jj=j0).
_J = np.arange(64)
_STEP_OF_J = ((_J >> 5) & 1) | (((_J >> 4) & 1) << 1) | (((_J >> 3) & 1) << 2) \
    | (((_J >> 2) & 1) << 3) | ((_J & 1) << 4)
_KS_OF_J = (_J >> 1) & 1


def _host_prep(core, losses, imbal, phases):
    """Per-core DRAM inputs.

    TKIN [128, (blk 5, j 64)] f16, blocks [t1, k1, t0, k0, t0].
    TRIG [128, (blk 8, j 64)] f16, blocks [ca, cb, sa, sb, cb, ca, sb, sa].
    Item j holds (pair = p + 128*ks(j), step = 32*core + step(j)).
    """
    i0 = CH * core
    a = 10.0 ** (-losses.astype(np.float64) / 20.0)
    tm = a * np.sqrt((1.0 + imbal.astype(np.float64)) * 0.5)
    km = a * np.sqrt((1.0 - imbal.astype(np.float64)) * 0.5)
    p = np.arange(128)[:, None]
    kp = p + 128 * _KS_OF_J[None, :]          # (128, 64) pair index
    ig = i0 + _STEP_OF_J[None, :] + 0 * kp    # (128, 64) global step
    t0 = tm[2 * ig, kp]
    k0 = km[2 * ig, kp]
    t1 = tm[2 * ig + 1, kp]
    k1 = km[2 * ig + 1, kp]
    TK = np.stack([t1, -k1, t0, k0, t0], axis=1)  # (128, 5, 64)
    al = phases.astype(np.float64)[ig, 2 * kp]
    be = phases.astype(np.float64)[ig, 2 * kp + 1]
    ca, cb, sa, sb = np.cos(al), np.cos(be), np.sin(al), np.sin(be)
    TR = np.stack([ca, cb, sa, sb, cb, ca, sb, sa], axis=1)  # (128, 8, 64)
    return (
        np.ascontiguousarray(TK.reshape(128, 320)).astype(np.float16),
        np.ascontiguousarray(TR.reshape(128, 512)).astype(np.float16),
    )


def _host_finish(Cs, phases_in, phases_out):
    """Multiply per-core partials, apply heaters, scatter the 2x2 blocks."""
    M = np.tile(np.eye(2, dtype=np.complex128), (NPAIR, 1, 1))
    for c in range(NCORE):
        # out [128, (c8, it)]: c8 outer, 8 items it = (i3, ks, i4)
        v = Cs[c].astype(np.float64).reshape(128, 2, 2, 2, 2, 2, 2)  # p,q,r,s,i3,ks,i4
        P = v[:, 0] + 1j * v[:, 1]                                   # p,r,s,i3,ks,i4
        Pc = np.empty((NPAIR, 2, 2, 2, 2), np.complex128)            # k,i4,i3,r,s
        Pc[0:128] = P[:, :, :, :, 0, :].transpose(0, 4, 3, 1, 2)
        Pc[128:256] = P[:, :, :, :, 1, :].transpose(0, 4, 3, 1, 2)
        M = Pc[:, 1, 1] @ (Pc[:, 1, 0] @ (Pc[:, 0, 1] @ (Pc[:, 0, 0] @ M)))
    ei = np.exp(1j * phases_in.astype(np.float64)).reshape(NPAIR, 2)
    eo = np.exp(1j * phases_out.astype(np.float64)).reshape(NPAIR, 2)
    G = (eo[:, :, None] * M * ei[:, None, :]).astype(np.complex64)
    out = np.zeros((N, N), np.complex64)
    idx = np.arange(NPAIR) * 2
    out[idx, idx] = G[:, 0, 0]
    out[idx, idx + 1] = G[:, 0, 1]
    out[idx + 1, idx] = G[:, 1, 0]
    out[idx + 1, idx + 1] = G[:, 1, 1]
    return out


# ---------------------------------------------------------------------------
# bass module
# ---------------------------------------------------------------------------

_NC = None


def _build_module():
    import concourse.bass as bass
    import concourse.bacc as bacc
    import concourse.mybir as mybir
    from concourse import tile

    f16 = mybir.dt.float16
    OP = mybir.AluOpType

    nc = bacc.Bacc("TRN2", target_bir_lowering=False, debug=False, num_devices=NCORE)
    tk_ext = nc.dram_tensor("tkin", [128, 320], f16, kind="ExternalInput").ap()
    tr_ext = nc.dram_tensor("trig", [128, 512], f16, kind="ExternalInput").ap()
    out_ext = nc.dram_tensor("out", [128, 64], f16, kind="ExternalOutput").ap()

    with tile.TileContext(nc) as tc:
        with tc.tile_pool(name="sbuf", bufs=1) as pool:
            tkin = pool.tile([128, 320], f16)
            trig = pool.tile([128, 512], f16)
            nc.sync.dma_start(tkin[:], tk_ext[:])
            nc.scalar.dma_start(trig[:], tr_ext[:])

            tkv = tkin[:].rearrange("p (b f) -> p b f", b=5)      # f = items 64
            trv = trig[:].rearrange("p (b f) -> p b f", b=8)

            # coef: Ce = t1*(t0,k0) -> (tt, tk);  Co = k1*(k0, t0) -> (kk, kt)
            ce = pool.tile([128, 128], f16)   # [j2, f]
            co = pool.tile([128, 128], f16)
            cev = ce[:].rearrange("p (j f) -> p j f", j=2)
            cov = co[:].rearrange("p (j f) -> p j f", j=2)
            nc.vector.tensor_mul(
                cev, tkv[:, 0].unsqueeze(1).broadcast_to((128, 2, 64)), tkv[:, 2:4])
            nc.vector.tensor_mul(
                cov, tkv[:, 1].unsqueeze(1).broadcast_to((128, 2, 64)), tkv[:, 3:5])

            # P_e[j,(w x),f] = Ce[j]*trig[w,x];  P_o = Co[j]*trig_sw[w,x]
            pe = pool.tile([128, 512], f16)
            po = pool.tile([128, 512], f16)
            for ptile, cv, tlo in ((pe, cev, 0), (po, cov, 4)):
                nc.vector.tensor_mul(
                    ptile[:].rearrange("p (j wx f) -> p j wx f", j=2, wx=4),
                    cv.unsqueeze(2).broadcast_to((128, 2, 4, 64)),
                    trv[:, tlo:tlo + 4].unsqueeze(1).broadcast_to((128, 2, 4, 64)),
                )

            # leaf comps L [128, (c8, item)] -- c8 outer.  Host sends -k1,
            # so po = (-kk, -kt) products and all three combines are plain
            # packed-f16 adds/subs (2x mode):
            #   op1 (j0):        S = pe + po'          c8 {0,3,4,7}
            #   op2 (j1, w=c):   S = pe - po'          c8 {5, 6}
            #   op3 (j1, w=s):   S = po' - pe          c8 {1, 2}
            L = pool.tile([128, 512], f16)
            pev = pe[:].rearrange("p (j wx f) -> p j wx f", j=2, wx=4)
            pov = po[:].rearrange("p (j wx f) -> p j wx f", j=2, wx=4)
            Lw = L[:].rearrange("p (w c2 f) -> p w c2 f", w=2, c2=4)
            nc.vector.tensor_add(
                Lw[:, :, 0::3, :],
                pev[:, 0].rearrange("p (w x) f -> p w x f", w=2),
                pov[:, 0].rearrange("p (w x) f -> p w x f", w=2))
            Lc8 = L[:].rearrange("p (c f) -> p c f", c=8)
            nc.vector.tensor_sub(Lc8[:, 5:7, :], pev[:, 1, 0:2], pov[:, 1, 0:2])
            nc.vector.tensor_sub(Lc8[:, 1:3, :], pov[:, 1, 2:4], pev[:, 1, 2:4])

            # ---- tree: 4 levels, half-pairing (j, j+I/2), all item-innermost
            cur = L
            n = 32
            while n >= 8:
                Tc = cur[:].rearrange("p (c it) -> p c it", c=8)
                Aq = Tc[:, :, n:]    # later-step items: left factor
                Bq = Tc[:, :, :n]
                Ax = Aq.rearrange("p (qar m) it -> p qar m it", m=2)
                Bx = Bq.rearrange("p (qbm s) it -> p qbm s it", s=2)
                M = pool.tile([128, 32 * n], f16, name=f"M{n}")
                Mv = M[:].rearrange("p (qb m qar s n) -> p qb m qar s n",
                                    qb=2, m=2, qar=4, s=2)
                for qb in range(2):
                    for m in range(2):
                        nc.vector.tensor_mul(
                            Mv[:, qb, m],
                            Ax[:, :, m, :].unsqueeze(2)
                            .broadcast_to((128, 4, 2, n)),
                            Bx[:, 2 * qb + m].unsqueeze(1)
                            .broadcast_to((128, 4, 2, n)),
                        )
                # U = M[qb0,:,qa0]-M[qb1,:,qa1]; V = M[qb1,:,qa0]+M[qb0,:,qa1]
                Mq = M[:].rearrange("p (qb m qa rs n) -> p qb m qa rs n",
                                    qb=2, m=2, qa=2, rs=4)
                W = pool.tile([128, 16 * n], f16, name=f"W{n}")
                Wv = W[:].rearrange("p (t m rs n) -> p t m rs n", t=2, m=2, rs=4)
                nc.vector.tensor_sub(Wv[:, 0], Mq[:, 0, :, 0], Mq[:, 1, :, 1])
                nc.vector.tensor_add(Wv[:, 1], Mq[:, 1, :, 0], Mq[:, 0, :, 1])
                nxt = pool.tile([128, 8 * n], f16, name=f"L{n}")
                nxv = nxt[:].rearrange("p (q rs it) -> p q rs it", q=2, rs=4)
                nc.vector.tensor_add(nxv, Wv[:, :, 0], Wv[:, :, 1])
                cur = nxt
                n //= 2

            nc.sync.dma_start(out_ext[:], cur[:])

    nc.finalize()
    return nc


def _get_module():
    global _NC
    if _NC is None:
        _NC = _build_module()
    return _NC


def kernel(ht_in_phase, ht_out_phase, ht_full_phases, mmi_i_losses, mmi_imbalances):
    from concourse.bass_utils import run_bass_kernel_spmd

    nc = _get_module()
    losses = np.asarray(mmi_i_losses, np.float32)
    imbal = np.asarray(mmi_imbalances, np.float32)
    phases = np.asarray(ht_full_phases, np.float32)
    in_maps = []
    for c in range(NCORE):
        TK, TR = _host_prep(c, losses, imbal, phases)
        in_maps.append({"tkin": TK, "trig": TR})
    res = run_bass_kernel_spmd(nc, in_maps, list(range(NCORE)))
    Cs = [res.results[c]["out"] for c in range(NCORE)]
    return _host_finish(
        Cs, np.asarray(ht_in_phase, np.float32), np.asarray(ht_out_phase, np.float32)
    )
